# revision 5
# baseline (speedup 1.0000x reference)
"""Trainium2 Bass kernel for a custom-activation LSTM cell.

  gates = (x @ w_ih.T + b_ih) + (h @ w_hh.T + b_hh)   # [B, 4H], gate order f,i,ic,o
  ft, it, ot = sigmoid(...), i_cands = sin(ic_in)
  ct = c*ft + sin(ic_in)*it ; ht = sigmoid(o_in)*sin(ct)

Sharding: each of the 8 cores computes the SAME 256-wide slice of H for all
four gates (rows g*2048 + core*256 .. +256 of the weight matrices). Each core
then owns columns [core*256, (core+1)*256) of ht/ct for the full batch — no
cross-core communication is needed.

Layout: compute is done transposed — out[g_tile, batch] = wT.T @ actT — so the
host pre-transposes x/h/c and the per-core weight slices once (cheap, shared
across cores for x/h/c), and the device kernel does only contiguous DMA.

Matmuls run as float32r (single-pass fp32, full PE rate at moving dim >= 256,
fp32 PSUM accumulate). sigmoid is computed as 0.5*tanh(0.5x + 0.5b) + 0.5 so
that tanh and sin come from ONE ACT table set (silu_and_others) — a raw
sigmoid would force a ~2.7us table switch per tile against sin.
"""

import numpy as np

import concourse.bass as bass
import concourse.tile as tile
from concourse import bacc, mybir
from concourse.bass_utils import run_bass_kernel_spmd

# Problem shapes (hardcoded per the harness contract).
B, IN, H = 4096, 1024, 2048
NCORES = 8
P = 128
SH = H // NCORES          # 256  H-slice per core
G = 4 * SH                # 1024 gate rows per core (f,i,ic,o x 256)
MT = G // P               # 8 m-tiles: [f0 f1 i0 i1 c0 c1 o0 o1]
KX = IN // P              # 8 k-tiles from x
KH = H // P               # 16 k-tiles from h
NB = 256                  # batch chunk (matmul moving dim)
NBCH = B // NB            # 16 chunks

F32 = mybir.dt.float32
F32R = mybir.dt.float32r
ACT = mybir.ActivationFunctionType

_MODULES: dict[int, "bacc.Bacc"] = {}


def _build_module(repeats: int = 1, lead_silu: bool = True) -> "bacc.Bacc":
    """Build + compile the per-core Bass module.

    repeats > 1 wraps the whole compute in a hardware loop (used only for
    timing: the per-iteration device time is (T(R) - T(1)) / (R - 1))."""
    nc = bacc.Bacc("TRN2", target_bir_lowering=False, debug=False,
                   num_devices=NCORES)

    xT = nc.dram_tensor("xT", [IN, B], F32R, kind="ExternalInput").ap()
    hT = nc.dram_tensor("hT", [H, B], F32R, kind="ExternalInput").ap()
    cT = nc.dram_tensor("cT", [SH, B], F32, kind="ExternalInput").ap()
    wih = nc.dram_tensor("wih", [IN, G], F32R, kind="ExternalInput").ap()
    whh = nc.dram_tensor("whh", [H, G], F32R, kind="ExternalInput").ap()
    biasd = nc.dram_tensor("biasd", [P, MT], F32, kind="ExternalInput").ap()
    htT = nc.dram_tensor("htT", [SH, B], F32, kind="ExternalOutput").ap()
    ctT = nc.dram_tensor("ctT", [SH, B], F32, kind="ExternalOutput").ap()

    xT3 = xT.rearrange("(ko p) b -> p ko b", p=P)      # [128, KX, B]
    hT3 = hT.rearrange("(ko p) b -> p ko b", p=P)      # [128, KH, B]
    cT3 = cT.rearrange("(po p) b -> p po b", p=P)      # [128, 2, B]
    wih3 = wih.rearrange("(ko p) g -> p ko g", p=P)    # [128, KX, G]
    whh3 = whh.rearrange("(ko p) g -> p ko g", p=P)    # [128, KH, G]
    htT3 = htT.rearrange("(po p) b -> p po b", p=P)
    ctT3 = ctT.rearrange("(po p) b -> p po b", p=P)

    with tile.TileContext(nc) as tc:
        with (
            tc.tile_pool(name="wpool", bufs=1) as wpool,
            tc.tile_pool(name="apool", bufs=2) as apool,
            tc.tile_pool(name="gpool", bufs=2) as gpool,
            tc.tile_pool(name="opool", bufs=3) as opool,
            tc.tile_pool(name="pspool", bufs=8, space="PSUM") as pspool,
        ):
            # Weights + bias resident in SBUF for the whole kernel.
            w_ih_sb = wpool.tile([P, KX, G], F32R)
            nc.sync.dma_start(out=w_ih_sb, in_=wih3)
            w_hh_sb = wpool.tile([P, KH, G], F32R)
            nc.sync.dma_start(out=w_hh_sb, in_=whh3)
            bias_sb = wpool.tile([P, MT], F32)
            nc.sync.dma_start(out=bias_sb, in_=biasd)

            # Dummy Silu: forces the ACT table loader to pick the
            # silu_and_others set (the only one containing BOTH Tanh and
            # Sin), so the whole kernel needs exactly one table load.
            # Without it the loader ping-pongs exp_and_others (Tanh) and
            # trig_and_small (Sin) at ~2.7us per switch, 64 times.
            if lead_silu:
                dummy = wpool.tile([P, 1], F32)
                nc.vector.memset(dummy, 0.0)
                nc.scalar.activation(dummy, dummy, ACT.Silu)

            def body():
                for nb in range(NBCH):
                    bsl = bass.ds(nb * NB, NB)
                    xc = apool.tile([P, KX, NB], F32R, tag="xc")
                    nc.sync.dma_start(out=xc, in_=xT3[:, :, bsl])
                    hc = apool.tile([P, KH, NB], F32R, tag="hc")
                    nc.sync.dma_start(out=hc, in_=hT3[:, :, bsl])
                    cc = apool.tile([P, 2, NB], F32, tag="cc")
                    nc.sync.dma_start(out=cc, in_=cT3[:, :, bsl])

                    for ph in range(2):  # H-slice half (two 128-row m-tiles)
                        ps = []
                        for gi in range(4):  # f, i, ic, o
                            m = 2 * gi + ph
                            gsl = bass.ds(m * P, P)
                            pt = pspool.tile([P, NB], F32, tag="ps")
                            for k in range(KX):
                                nc.tensor.matmul(
                                    pt,
                                    lhsT=w_ih_sb[:, k, gsl],
                                    rhs=xc[:, k, :],
                                    start=(k == 0), stop=False,
                                )
                            for k in range(KH):
                                nc.tensor.matmul(
                                    pt,
                                    lhsT=w_hh_sb[:, k, gsl],
                                    rhs=hc[:, k, :],
                                    start=False, stop=(k == KH - 1),
                                )
                            ps.append(pt)

                        cols = [2 * gi + ph for gi in range(4)]
                        PI, TWO_PI = float(np.pi), float(2 * np.pi)
                        ft = gpool.tile([P, NB], F32, tag="ft")
                        it = gpool.tile([P, NB], F32, tag="it")
                        gt = gpool.tile([P, NB], F32, tag="gt")
                        ot = gpool.tile([P, NB], F32, tag="ot")
                        # sigmoid(z+b) = 0.5*tanh(0.5z + 0.5b) + 0.5
                        # (bias column for tanh gates is pre-scaled by 0.5)
                        nc.scalar.activation(ft, ps[0], ACT.Tanh,
                                             bias=bias_sb[:, cols[0]:cols[0] + 1],
                                             scale=0.5)
                        nc.scalar.activation(it, ps[1], ACT.Tanh,
                                             bias=bias_sb[:, cols[1]:cols[1] + 1],
                                             scale=0.5)
                        # ACT Sin is only valid on [-pi, pi]; add the bias and
                        # wrap by one 2*pi period on the DVE first.
                        gw = gpool.tile([P, NB], F32, tag="gw")
                        nc.vector.add_range_wrap(
                            gw, ps[2], bias_sb[:, cols[2]:cols[2] + 1], PI, TWO_PI)
                        nc.scalar.activation(gt, gw, ACT.Sin)
                        nc.scalar.activation(ot, ps[3], ACT.Tanh,
                                             bias=bias_sb[:, cols[3]:cols[3] + 1],
                                             scale=0.5)
                        MUL, ADD = mybir.AluOpType.mult, mybir.AluOpType.add
                        nc.vector.tensor_scalar(ft, ft, 0.5, 0.5, MUL, ADD)
                        nc.vector.tensor_scalar(it, it, 0.5, 0.5, MUL, ADD)
                        nc.vector.tensor_scalar(ot, ot, 0.5, 0.5, MUL, ADD)

                        ctn = opool.tile([P, NB], F32, tag="ctn")
                        tmp = opool.tile([P, NB], F32, tag="tmp")
                        nc.vector.tensor_mul(ctn, cc[:, ph, :], ft)
                        nc.vector.tensor_mul(tmp, gt, it)
                        nc.vector.tensor_add(ctn, ctn, tmp)
                        cw = opool.tile([P, NB], F32, tag="cw")
                        nc.vector.add_range_wrap(cw, ctn, 0.0, PI, TWO_PI)
                        sct = opool.tile([P, NB], F32, tag="sct")
                        nc.scalar.activation(sct, cw, ACT.Sin)
                        htn = opool.tile([P, NB], F32, tag="htn")
                        nc.vector.tensor_mul(htn, ot, sct)
                        nc.sync.dma_start(out=ctT3[:, ph, bsl], in_=ctn)
                        nc.sync.dma_start(out=htT3[:, ph, bsl], in_=htn)

            if repeats == 1:
                body()
            else:
                with tc.For_i(0, repeats, 1):
                    body()

    nc.compile()
    return nc


def _get_module(repeats: int = 1) -> "bacc.Bacc":
    if repeats not in _MODULES:
        _MODULES[repeats] = _build_module(repeats)
    return _MODULES[repeats]


def make_in_maps(x, h, c, w_ih, w_hh, b_ih, b_hh):
    """Host-side shard + transpose. Returns the per-core input maps."""
    x = np.asarray(x, np.float32)
    h = np.asarray(h, np.float32)
    c = np.asarray(c, np.float32)
    w_ih = np.asarray(w_ih, np.float32)
    w_hh = np.asarray(w_hh, np.float32)
    bias = np.asarray(b_ih, np.float32) + np.asarray(b_hh, np.float32)

    xT = np.ascontiguousarray(x.T)          # [IN, B] shared by all cores
    hTt = np.ascontiguousarray(h.T)         # [H, B]
    cTt = np.ascontiguousarray(c.T)         # [H, B]

    # m-tile scale: 0.5 for tanh-based sigmoid gates (f,i,o), 1.0 for sin (ic)
    mscale = np.array([0.5, 0.5, 0.5, 0.5, 1.0, 1.0, 0.5, 0.5], np.float32)

    in_maps = []
    for core in range(NCORES):
        rows = np.concatenate(
            [gate * H + core * SH + np.arange(SH) for gate in range(4)])
        wih_c = np.ascontiguousarray(w_ih[rows].T)   # [IN, G]
        whh_c = np.ascontiguousarray(w_hh[rows].T)   # [H, G]
        b_c = bias[rows]                             # [G]
        bias_mat = np.ascontiguousarray(
            (b_c.reshape(MT, P) * mscale[:, None]).T)  # [P, MT]
        in_maps.append({
            "xT": xT,
            "hT": hTt,
            "cT": np.ascontiguousarray(cTt[core * SH:(core + 1) * SH]),
            "wih": wih_c,
            "whh": whh_c,
            "biasd": bias_mat,
        })
    return in_maps


def assemble_outputs(results):
    """results: per-core dicts with htT/ctT [SH, B] -> full (ht, ct)."""
    htT = np.concatenate([results[c]["htT"] for c in range(NCORES)], axis=0)
    ctT = np.concatenate([results[c]["ctT"] for c in range(NCORES)], axis=0)
    ht = np.ascontiguousarray(htT.T)
    ct = np.ascontiguousarray(ctT.T)
    return ht, ct


def kernel(x, h, c, w_ih, w_hh, b_ih, b_hh):
    nc = _get_module(repeats=1)
    in_maps = make_in_maps(x, h, c, w_ih, w_hh, b_ih, b_hh)
    res = run_bass_kernel_spmd(nc, in_maps, core_ids=list(range(NCORES)))
    return assemble_outputs(res.results)


# revision 6
# speedup vs baseline: 50.7466x; 50.7466x over previous
"""Trainium2 Bass kernel for a custom-activation LSTM cell.

  gates = (x @ w_ih.T + b_ih) + (h @ w_hh.T + b_hh)   # [B, 4H], gate order f,i,ic,o
  ft, it, ot = sigmoid(...), i_cands = sin(ic_in)
  ct = c*ft + sin(ic_in)*it ; ht = sigmoid(o_in)*sin(ct)

Sharding: each of the 8 cores computes the SAME 256-wide slice of H for all
four gates (rows g*2048 + core*256 .. +256 of the weight matrices). Each core
then owns columns [core*256, (core+1)*256) of ht/ct for the full batch — no
cross-core communication is needed.

Layout: compute is done transposed — out[g_tile, batch] = wT.T @ actT — so the
host pre-transposes x/h/c and the per-core weight slices once (cheap, shared
across cores for x/h/c), and the device kernel does only contiguous DMA.

Matmuls run as float32r (single-pass fp32, full PE rate at moving dim >= 256,
fp32 PSUM accumulate). sigmoid is computed as 0.5*tanh(0.5x + 0.5b) + 0.5 so
that tanh and sin come from ONE ACT table set (silu_and_others) — a raw
sigmoid would force a ~2.7us table switch per tile against sin.
"""

import numpy as np

import concourse.bass as bass
import concourse.tile as tile
from concourse import bacc, mybir
from concourse.bass_utils import run_bass_kernel_spmd

# Problem shapes (hardcoded per the harness contract).
B, IN, H = 4096, 1024, 2048
NCORES = 8
P = 128
SH = H // NCORES          # 256  H-slice per core
G = 4 * SH                # 1024 gate rows per core (f,i,ic,o x 256)
MT = G // P               # 8 m-tiles: [f0 f1 i0 i1 c0 c1 o0 o1]
KX = IN // P              # 8 k-tiles from x
KH = H // P               # 16 k-tiles from h
NB = 256                  # batch chunk (matmul moving dim)
NBCH = B // NB            # 16 chunks

F32 = mybir.dt.float32
F32R = mybir.dt.float32r
ACT = mybir.ActivationFunctionType

_MODULES: dict[int, "bacc.Bacc"] = {}


def _build_module(repeats: int = 1, lead_silu: bool = True,
                  internal_io: bool = False) -> "bacc.Bacc":
    """Build + compile the per-core Bass module.

    repeats > 1 wraps the whole compute in a hardware loop (used only for
    timing: the per-iteration device time is (T(R) - T(1)) / (R - 1))."""
    nc = bacc.Bacc("TRN2", target_bir_lowering=False, debug=False,
                   num_devices=NCORES)

    # internal_io=True is a timing-only variant: the big tensors live in
    # Internal DRAM (uninitialized, never uploaded/downloaded) so the
    # per-call wall time is not dominated by host<->device transfers.
    kin = "Internal" if internal_io else "ExternalInput"
    kout = "Internal" if internal_io else "ExternalOutput"

    xT = nc.dram_tensor("xT", [IN, B], F32R, kind=kin).ap()
    hT = nc.dram_tensor("hT", [H, B], F32R, kind=kin).ap()
    cT = nc.dram_tensor("cT", [SH, B], F32, kind=kin).ap()
    wih = nc.dram_tensor("wih", [IN, G], F32R, kind=kin).ap()
    whh = nc.dram_tensor("whh", [H, G], F32R, kind=kin).ap()
    biasd = nc.dram_tensor("biasd", [P, MT], F32, kind="ExternalInput").ap()
    htT = nc.dram_tensor("htT", [SH, B], F32, kind=kout).ap()
    ctT = nc.dram_tensor("ctT", [SH, B], F32, kind=kout).ap()

    xT3 = xT.rearrange("(ko p) b -> p ko b", p=P)      # [128, KX, B]
    hT3 = hT.rearrange("(ko p) b -> p ko b", p=P)      # [128, KH, B]
    cT3 = cT.rearrange("(po p) b -> p po b", p=P)      # [128, 2, B]
    wih3 = wih.rearrange("(ko p) g -> p ko g", p=P)    # [128, KX, G]
    whh3 = whh.rearrange("(ko p) g -> p ko g", p=P)    # [128, KH, G]
    htT3 = htT.rearrange("(po p) b -> p po b", p=P)
    ctT3 = ctT.rearrange("(po p) b -> p po b", p=P)

    with tile.TileContext(nc) as tc:
        with (
            tc.tile_pool(name="wpool", bufs=1) as wpool,
            tc.tile_pool(name="apool", bufs=2) as apool,
            tc.tile_pool(name="gpool", bufs=2) as gpool,
            tc.tile_pool(name="opool", bufs=3) as opool,
            tc.tile_pool(name="pspool", bufs=8, space="PSUM") as pspool,
        ):
            # Weights + bias resident in SBUF for the whole kernel.
            w_ih_sb = wpool.tile([P, KX, G], F32R)
            nc.sync.dma_start(out=w_ih_sb, in_=wih3)
            w_hh_sb = wpool.tile([P, KH, G], F32R)
            nc.sync.dma_start(out=w_hh_sb, in_=whh3)
            bias_sb = wpool.tile([P, MT], F32)
            nc.sync.dma_start(out=bias_sb, in_=biasd)

            # Dummy Silu: forces the ACT table loader to pick the
            # silu_and_others set (the only one containing BOTH Tanh and
            # Sin), so the whole kernel needs exactly one table load.
            # Without it the loader ping-pongs exp_and_others (Tanh) and
            # trig_and_small (Sin) at ~2.7us per switch, 64 times.
            if lead_silu:
                dummy = wpool.tile([P, 1], F32)
                nc.vector.memset(dummy, 0.0)
                nc.scalar.activation(dummy, dummy, ACT.Silu)

            def body():
                for nb in range(NBCH):
                    bsl = bass.ds(nb * NB, NB)
                    xc = apool.tile([P, KX, NB], F32R, tag="xc")
                    nc.sync.dma_start(out=xc, in_=xT3[:, :, bsl])
                    hc = apool.tile([P, KH, NB], F32R, tag="hc")
                    nc.sync.dma_start(out=hc, in_=hT3[:, :, bsl])
                    cc = apool.tile([P, 2, NB], F32, tag="cc")
                    nc.sync.dma_start(out=cc, in_=cT3[:, :, bsl])

                    for ph in range(2):  # H-slice half (two 128-row m-tiles)
                        ps = []
                        for gi in range(4):  # f, i, ic, o
                            m = 2 * gi + ph
                            gsl = bass.ds(m * P, P)
                            pt = pspool.tile([P, NB], F32, tag="ps")
                            for k in range(KX):
                                nc.tensor.matmul(
                                    pt,
                                    lhsT=w_ih_sb[:, k, gsl],
                                    rhs=xc[:, k, :],
                                    start=(k == 0), stop=False,
                                )
                            for k in range(KH):
                                nc.tensor.matmul(
                                    pt,
                                    lhsT=w_hh_sb[:, k, gsl],
                                    rhs=hc[:, k, :],
                                    start=False, stop=(k == KH - 1),
                                )
                            ps.append(pt)

                        cols = [2 * gi + ph for gi in range(4)]
                        PI, TWO_PI = float(np.pi), float(2 * np.pi)
                        ft = gpool.tile([P, NB], F32, tag="ft")
                        it = gpool.tile([P, NB], F32, tag="it")
                        gt = gpool.tile([P, NB], F32, tag="gt")
                        ot = gpool.tile([P, NB], F32, tag="ot")
                        # sigmoid(z+b) = 0.5*tanh(0.5z + 0.5b) + 0.5
                        # (bias column for tanh gates is pre-scaled by 0.5)
                        nc.scalar.activation(ft, ps[0], ACT.Tanh,
                                             bias=bias_sb[:, cols[0]:cols[0] + 1],
                                             scale=0.5)
                        nc.scalar.activation(it, ps[1], ACT.Tanh,
                                             bias=bias_sb[:, cols[1]:cols[1] + 1],
                                             scale=0.5)
                        # ACT Sin is only valid on [-pi, pi]; add the bias and
                        # wrap by one 2*pi period on the DVE first.
                        gw = gpool.tile([P, NB], F32, tag="gw")
                        nc.vector.add_range_wrap(
                            gw, ps[2], bias_sb[:, cols[2]:cols[2] + 1], PI, TWO_PI)
                        nc.scalar.activation(gt, gw, ACT.Sin)
                        nc.scalar.activation(ot, ps[3], ACT.Tanh,
                                             bias=bias_sb[:, cols[3]:cols[3] + 1],
                                             scale=0.5)
                        MUL, ADD = mybir.AluOpType.mult, mybir.AluOpType.add
                        nc.vector.tensor_scalar(ft, ft, 0.5, 0.5, MUL, ADD)
                        nc.vector.tensor_scalar(it, it, 0.5, 0.5, MUL, ADD)
                        nc.vector.tensor_scalar(ot, ot, 0.5, 0.5, MUL, ADD)

                        ctn = opool.tile([P, NB], F32, tag="ctn")
                        tmp = opool.tile([P, NB], F32, tag="tmp")
                        nc.vector.tensor_mul(ctn, cc[:, ph, :], ft)
                        nc.vector.tensor_mul(tmp, gt, it)
                        nc.vector.tensor_add(ctn, ctn, tmp)
                        cw = opool.tile([P, NB], F32, tag="cw")
                        nc.vector.add_range_wrap(cw, ctn, 0.0, PI, TWO_PI)
                        sct = opool.tile([P, NB], F32, tag="sct")
                        nc.scalar.activation(sct, cw, ACT.Sin)
                        htn = opool.tile([P, NB], F32, tag="htn")
                        nc.vector.tensor_mul(htn, ot, sct)
                        nc.sync.dma_start(out=ctT3[:, ph, bsl], in_=ctn)
                        nc.sync.dma_start(out=htT3[:, ph, bsl], in_=htn)

            if repeats == 1:
                body()
            else:
                with tc.For_i(0, repeats, 1):
                    body()

            if internal_io:
                done = nc.dram_tensor("done", [P, MT], F32,
                                      kind="ExternalOutput").ap()
                dtile = wpool.tile([P, MT], F32)
                nc.vector.tensor_copy(dtile, bias_sb)
                nc.sync.dma_start(out=done, in_=dtile)

    nc.compile()
    return nc


def _get_module(repeats: int = 1) -> "bacc.Bacc":
    if repeats not in _MODULES:
        _MODULES[repeats] = _build_module(repeats)
    return _MODULES[repeats]


def make_in_maps(x, h, c, w_ih, w_hh, b_ih, b_hh):
    """Host-side shard + transpose. Returns the per-core input maps."""
    x = np.asarray(x, np.float32)
    h = np.asarray(h, np.float32)
    c = np.asarray(c, np.float32)
    w_ih = np.asarray(w_ih, np.float32)
    w_hh = np.asarray(w_hh, np.float32)
    bias = np.asarray(b_ih, np.float32) + np.asarray(b_hh, np.float32)

    xT = np.ascontiguousarray(x.T)          # [IN, B] shared by all cores
    hTt = np.ascontiguousarray(h.T)         # [H, B]
    cTt = np.ascontiguousarray(c.T)         # [H, B]

    # m-tile scale: 0.5 for tanh-based sigmoid gates (f,i,o), 1.0 for sin (ic)
    mscale = np.array([0.5, 0.5, 0.5, 0.5, 1.0, 1.0, 0.5, 0.5], np.float32)

    in_maps = []
    for core in range(NCORES):
        rows = np.concatenate(
            [gate * H + core * SH + np.arange(SH) for gate in range(4)])
        wih_c = np.ascontiguousarray(w_ih[rows].T)   # [IN, G]
        whh_c = np.ascontiguousarray(w_hh[rows].T)   # [H, G]
        b_c = bias[rows]                             # [G]
        bias_mat = np.ascontiguousarray(
            (b_c.reshape(MT, P) * mscale[:, None]).T)  # [P, MT]
        in_maps.append({
            "xT": xT,
            "hT": hTt,
            "cT": np.ascontiguousarray(cTt[core * SH:(core + 1) * SH]),
            "wih": wih_c,
            "whh": whh_c,
            "biasd": bias_mat,
        })
    return in_maps


def assemble_outputs(results):
    """results: per-core dicts with htT/ctT [SH, B] -> full (ht, ct)."""
    htT = np.concatenate([results[c]["htT"] for c in range(NCORES)], axis=0)
    ctT = np.concatenate([results[c]["ctT"] for c in range(NCORES)], axis=0)
    ht = np.ascontiguousarray(htT.T)
    ct = np.ascontiguousarray(ctT.T)
    return ht, ct


def kernel(x, h, c, w_ih, w_hh, b_ih, b_hh):
    nc = _get_module(repeats=1)
    in_maps = make_in_maps(x, h, c, w_ih, w_hh, b_ih, b_hh)
    res = run_bass_kernel_spmd(nc, in_maps, core_ids=list(range(NCORES)))
    return assemble_outputs(res.results)
